# revision 11
# baseline (speedup 1.0000x reference)
"""Child-Sum TreeLSTM over a perfect binary tree (N=65535, depth 15) on 8 trn2 cores.

Sharding: each core owns one depth-3 subtree (levels 15..3 are fully local:
children of node range [a,b) are [2a+1,2b+1), so per-core level slices are
contiguous and child gathers are stride-2 local reads).  The 8 subtree roots
are AllGathered (4 KB) and the top 7 nodes are computed redundantly on every
core; the host takes them from core 0.

On-chip layout is feature-major ([feature-chunk=128 partitions, nodes free]);
the host pre-transposes the inputs so the device never transposes anything.
Biases are folded into the x-side matmul via an appended ones-row (K=301).
Matmuls run in bf16 (fp32 PSUM accumulation); gates/cell state are fp32.
"""

import sys

sys.path.insert(0, "/opt/trn_rl_repo")

import numpy as np
import ml_dtypes

IN_DIM = 300
MEM = 256
DEPTH = 15
N_NODES = 2 ** (DEPTH + 1) - 1  # 65535
NCORES = 8
SUB_DEPTH = 3  # shard at depth 3 -> 8 subtrees
NB = 512  # node block size (one psum bank per 128-feature chunk)

# per-core level sizes for the sharded levels, leaves first
LEVELS = list(range(DEPTH, SUB_DEPTH - 1, -1))  # 15..3
S_OF = {d: 2 ** (d - SUB_DEPTH) for d in LEVELS}  # 4096..1
OFF_OF = {}
_off = 0
for _d in LEVELS:
    OFF_OF[_d] = _off
    _off += S_OF[_d]
N_LOCAL = _off  # 8191
TOP_COL0 = N_LOCAL  # columns 8191..8197 hold x of global nodes 0..6
N_COLS = N_LOCAL + 7  # 8198

F32 = None  # filled lazily (mybir dtype handles)
BF16 = None

_CACHE = {}


def _ceil_div(a, b):
    return -(-a // b)


def _build_program():
    import concourse.bass as bass
    import concourse.mybir as mybir
    import concourse.bacc as bacc
    from concourse import tile

    f32 = mybir.dt.float32
    bf16 = mybir.dt.bfloat16
    SIG = mybir.ActivationFunctionType.Sigmoid
    TANH = mybir.ActivationFunctionType.Tanh

    nc = bacc.Bacc("TRN2", target_bir_lowering=False, debug=False, num_devices=NCORES)

    xt = nc.dram_tensor("xt", [IN_DIM + 1, N_COLS], bf16, kind="ExternalInput")
    wx = nc.dram_tensor("wx", [IN_DIM + 1, 4 * MEM], bf16, kind="ExternalInput")
    wh = nc.dram_tensor("wh", [MEM, 3 * MEM], bf16, kind="ExternalInput")
    wf = nc.dram_tensor("wf", [MEM, MEM], bf16, kind="ExternalInput")
    out = nc.dram_tensor("out", [2, 2, 128, N_COLS], f32, kind="ExternalOutput")

    KCH = [(0, 128), (128, 128), (256, IN_DIM + 1 - 256)]  # k chunks of 301

    with tile.TileContext(nc) as tc:
        with (
            tc.tile_pool(name="const", bufs=1) as cpool,
            tc.tile_pool(name="perst", bufs=1) as ppool,
            tc.tile_pool(name="xp", bufs=3) as xpool,
            tc.tile_pool(name="wk", bufs=2) as wk,
            tc.tile_pool(name="hout", bufs=3) as hop,
            tc.tile_pool(name="ps", bufs=2, space="PSUM") as psp,
            tc.tile_pool(name="dram", bufs=1, space="DRAM") as dram,
        ):
            # ---- load weights ----
            wx_sb = []
            for i, (k0, kn) in enumerate(KCH):
                t = cpool.tile([kn, 4 * MEM], bf16, tag=f"wx{i}", name=f"wx{i}")
                nc.sync.dma_start(t[:], wx[k0 : k0 + kn, :])
                wx_sb.append(t)
            wh_sb = []
            for i in range(2):
                t = cpool.tile([128, 3 * MEM], bf16, tag=f"wh{i}", name=f"wh{i}")
                nc.sync.dma_start(t[:], wh[i * 128 : (i + 1) * 128, :])
                wh_sb.append(t)
            wf_sb = []
            for i in range(2):
                t = cpool.tile([128, MEM], bf16, tag=f"wf{i}", name=f"wf{i}")
                nc.sync.dma_start(t[:], wf[i * 128 : (i + 1) * 128, :])
                wf_sb.append(t)

            # ---- persistent level buffers (A = odd levels, B = even) ----
            hbuf = {
                1: ppool.tile([128, 2, 4096], bf16, tag="hA", name="hA"),
                0: ppool.tile([128, 2, 2048], bf16, tag="hB", name="hB"),
            }
            cbuf = {
                1: ppool.tile([128, 2, 4096], f32, tag="cA", name="cA"),
                0: ppool.tile([128, 2, 2048], f32, tag="cB", name="cB"),
            }
            fbuf = {
                1: ppool.tile([128, 2, 4096], bf16, tag="fA", name="fA"),
                0: ppool.tile([128, 2, 2048], bf16, tag="fB", name="fB"),
            }

            def load_x(col0, s):
                """DMA x block [301, s] as 3 k-chunk tiles."""
                ts = []
                for i, (k0, kn) in enumerate(KCH):
                    t = xpool.tile([kn, NB], bf16, tag=f"xk{i}", name=f"xk{i}", bufs=2)
                    nc.sync.dma_start(t[:, :s], xt[k0 : k0 + kn, col0 : col0 + s])
                    ts.append(t)
                return ts

            def do_level(
                S,
                x_col0,
                out_col0,
                h_child,  # bf16 AP [128, 2, 2S] or None for leaves
                c_child,  # f32 AP [128, 2, 2S] or None
                f_child,  # bf16 AP [128, 2, 2S] (fpre of children) or None
                h_dest,  # bf16 AP [128, 2, >=S]
                c_dest,  # f32 AP [128, 2, >=S]
                fpre_out,  # bf16 AP [128, 2, >=S] or None
                root_sink=None,  # (gin dram tile,) to stash root c/h when S==1
            ):
                leaf = h_child is None
                explicit_f = (not leaf) and (f_child is None)
                nblk = _ceil_div(S, NB)
                for b in range(nblk):
                    col = b * NB
                    s = min(NB, S - col)
                    xts = load_x(x_col0 + col, s)
                    # two psum tiles per block; every 512-f32 chunk is one
                    # full 2KB bank = one accumulation zero-region
                    psA = psp.tile([128, 4, NB], f32, tag="ps", name="psA")
                    psB = psp.tile([128, 4, NB], f32, tag="ps", name="psB")

                    def pchunk(m):
                        return psA[:, m, :s] if m < 4 else psB[:, m - 4, :s]

                    n_m = 6 if leaf else 8
                    # x-side matmuls (+ folded bias)
                    for m in range(n_m):
                        msl = slice(m * 128, (m + 1) * 128)
                        for ki in range(3):
                            nc.tensor.matmul(
                                pchunk(m),
                                wx_sb[ki][:, msl],
                                xts[ki][:, :s],
                                start=(ki == 0),
                                stop=(ki == 2 and (leaf or m >= 6)),
                            )
                    if not leaf:
                        # child h sum (stride-2 gather from child level)
                        hs = wk.tile([128, 2, NB], bf16, tag="hs", name="hs")
                        nc.vector.tensor_add(
                            hs[:, :, :s],
                            h_child[:, :, 2 * col : 2 * (col + s) : 2],
                            h_child[:, :, 2 * col + 1 : 2 * (col + s) : 2],
                        )
                        for m in range(6):
                            msl = slice(m * 128, (m + 1) * 128)
                            for hc in range(2):
                                nc.tensor.matmul(
                                    pchunk(m),
                                    wh_sb[hc][:, msl],
                                    hs[:, hc, :s],
                                    start=False,
                                    stop=(hc == 1),
                                )
                        # f-gate pre-activations: fpre(child) + fx
                        pre_f = wk.tile([128, 4, NB], bf16, tag="pre_f", name="pre_f")
                        if explicit_f:
                            # compact hl / hr, then W_fh matmuls (top levels only)
                            hlr = wk.tile(
                                [128, 2, 2, NB], bf16, tag="hlr", name="hlr", bufs=1
                            )
                            for side in range(2):
                                nc.vector.tensor_copy(
                                    hlr[:, side, :, :s],
                                    h_child[:, :, 2 * col + side : 2 * (col + s) : 2],
                                )
                            psf = psp.tile([128, 4, NB], f32, tag="ps", name="psf")
                            for side in range(2):
                                for m in range(2):
                                    for hc in range(2):
                                        nc.tensor.matmul(
                                            psf[:, 2 * side + m, :s],
                                            wf_sb[hc][:, m * 128 : (m + 1) * 128],
                                            hlr[:, side, hc, :s],
                                            start=(hc == 0),
                                            stop=(hc == 1),
                                        )
                            # DVE may read only one PSUM operand per op
                            fx_sb = wk.tile(
                                [128, 2, NB], f32, tag="fxs", name="fx_sb", bufs=1
                            )
                            nc.vector.tensor_copy(fx_sb[:, :, :s], psB[:, 2:4, :s])
                            for side in range(2):
                                nc.vector.tensor_add(
                                    pre_f[:, 2 * side : 2 * side + 2, :s],
                                    psf[:, 2 * side : 2 * side + 2, :s],
                                    fx_sb[:, :, :s],
                                )
                        else:
                            for side in range(2):
                                nc.vector.tensor_add(
                                    pre_f[:, 2 * side : 2 * side + 2, :s],
                                    f_child[:, :, 2 * col + side : 2 * (col + s) : 2],
                                    psB[:, 2:4, :s],
                                )
                    # ---- gates ----
                    sig_io = wk.tile([128, 4, NB], f32, tag="sig_io", name="sig_io")
                    u_t = wk.tile([128, 2, NB], f32, tag="u_t", name="u_t")
                    nc.scalar.activation(sig_io[:, :, :s], psA[:, 0:4, :s], SIG)
                    nc.scalar.activation(u_t[:, :, :s], psB[:, 0:2, :s], TANH)
                    cs = c_dest[:, :, col : col + s]
                    nc.vector.tensor_mul(cs, sig_io[:, 0:2, :s], u_t[:, :, :s])
                    if not leaf:
                        sig_f = wk.tile(
                            [128, 4, NB], f32, tag="sig_f", name="sig_f", bufs=1
                        )
                        nc.scalar.activation(sig_f[:, :, :s], pre_f[:, :, :s], SIG)
                        fc = wk.tile([128, 2, NB], f32, tag="fc", name="fc")
                        nc.vector.tensor_mul(
                            fc[:, :, :s],
                            sig_f[:, 0:2, :s],
                            c_child[:, :, 2 * col : 2 * (col + s) : 2],
                        )
                        nc.vector.tensor_add(cs, cs, fc[:, :, :s])
                        fc2 = wk.tile([128, 2, NB], f32, tag="fc", name="fc2")
                        nc.vector.tensor_mul(
                            fc2[:, :, :s],
                            sig_f[:, 2:4, :s],
                            c_child[:, :, 2 * col + 1 : 2 * (col + s) : 2],
                        )
                        nc.vector.tensor_add(cs, cs, fc2[:, :, :s])
                    tc_t = wk.tile([128, 2, NB], f32, tag="tc_t", name="tc_t", bufs=1)
                    nc.scalar.activation(tc_t[:, :, :s], cs, TANH)
                    hst = hop.tile([128, 2, NB], f32, tag="hst", name="hst")
                    nc.vector.tensor_mul(
                        hst[:, :, :s], sig_io[:, 2:4, :s], tc_t[:, :, :s]
                    )
                    nc.vector.tensor_copy(h_dest[:, :, col : col + s], hst[:, :, :s])
                    # outputs
                    for ch in range(2):
                        nc.sync.dma_start(
                            out[1, ch, :, out_col0 + col : out_col0 + col + s],
                            hst[:, ch, :s],
                        )
                    if root_sink is not None and S == 1:
                        gin = root_sink
                        for ch in range(2):
                            nc.sync.dma_start(
                                gin[ch * 128 : (ch + 1) * 128, 0:1], cs[:, ch, :]
                            )
                            nc.sync.dma_start(
                                gin[ch * 128 : (ch + 1) * 128, 1:2], hst[:, ch, 0:1]
                            )
                # c output for the whole level
                for ch in range(2):
                    nc.sync.dma_start(
                        out[0, ch, :, out_col0 : out_col0 + S], c_dest[:, ch, :S]
                    )
                # fpre pass (f-gate h-side matmuls for this level's nodes)
                if fpre_out is not None:
                    FB = 512
                    for fb in range(_ceil_div(S, FB)):
                        col = fb * FB
                        s = min(FB, S - col)
                        psf = psp.tile([128, 2, 512], f32, tag="ps")
                        for m in range(2):
                            for hc in range(2):
                                nc.tensor.matmul(
                                    psf[:, m, :s],
                                    wf_sb[hc][:, m * 128 : (m + 1) * 128],
                                    h_dest[:, hc, col : col + s],
                                    start=(hc == 0),
                                    stop=(hc == 1),
                                )
                        nc.vector.tensor_copy(
                            fpre_out[:, :, col : col + s], psf[:, :, :s]
                        )

            # ---- gather bounce buffers ----
            gin = dram.tile([256, 2], f32)
            gout = dram.tile([256 * NCORES, 2], f32)

            # ---- sharded levels 15..3 ----
            for d in LEVELS:
                S = S_OF[d]
                par = d & 1
                h_child = c_child = f_child = None
                if d < DEPTH:
                    Sc = S_OF[d + 1]
                    h_child = hbuf[1 - par][:, :, : 2 * S]
                    c_child = cbuf[1 - par][:, :, : 2 * S]
                    f_child = fbuf[1 - par][:, :, : 2 * S]
                do_level(
                    S,
                    OFF_OF[d],
                    OFF_OF[d],
                    h_child,
                    c_child,
                    f_child,
                    hbuf[par],
                    cbuf[par],
                    fbuf[par][:, :, :S] if d > SUB_DEPTH else None,
                    root_sink=gin if d == SUB_DEPTH else None,
                )

            # ---- allgather the 8 subtree roots ----
            nc.gpsimd.collective_compute(
                "AllGather",
                mybir.AluOpType.bypass,
                replica_groups=[list(range(NCORES))],
                ins=[gin.opt()],
                outs=[gout.opt()],
            )
            # load gathered roots feature-major: (p, ch, rank)
            g_ap = gout[:, :].rearrange("(r c p) t -> c p r t", p=128, c=2)
            c_top = ppool.tile([128, 2, 8], f32, tag="ctop")
            h_topf = ppool.tile([128, 2, 8], f32, tag="htopf")
            for ch in range(2):
                nc.sync.dma_start(c_top[:, ch, :], g_ap[ch, :, :, 0])
                nc.sync.dma_start(h_topf[:, ch, :], g_ap[ch, :, :, 1])
            h_top = ppool.tile([128, 2, 8], bf16, tag="htop")
            nc.vector.tensor_copy(h_top[:], h_topf[:])

            # ---- top levels 2..0 (computed redundantly on every core) ----
            prev_h, prev_c = h_top, c_top
            for d in (2, 1, 0):
                S = 2**d
                node0 = S - 1
                col0 = TOP_COL0 + node0
                h_d = ppool.tile([128, 2, S], bf16, tag=f"ht{d}", name=f"ht{d}")
                c_d = ppool.tile([128, 2, S], f32, tag=f"ct{d}", name=f"ct{d}")
                do_level(
                    S,
                    col0,
                    col0,
                    prev_h[:, :, : 2 * S],
                    prev_c[:, :, : 2 * S],
                    None,
                    h_d,
                    c_d,
                    None,
                )
                prev_h, prev_c = h_d, c_d

    nc.compile()
    return nc


def _get_program():
    if "nc" not in _CACHE:
        _CACHE["nc"] = _build_program()
    return _CACHE["nc"]


def _preprocess(inputs, W_ioux, b_ioux, W_iouh, b_iouh, W_fx, b_fx, W_fh, b_fh):
    """Build per-core input maps (numpy only)."""
    bf = ml_dtypes.bfloat16
    # x transposed + ones row, in per-core local column order
    wx_cat = np.concatenate([W_ioux, W_fx], axis=0)  # [1024, 300]
    b_cat = np.concatenate([b_ioux + b_iouh, b_fx + b_fh], axis=0)  # [1024]
    wx_full = np.concatenate([wx_cat.T, b_cat[None, :]], axis=0)  # [301, 1024]
    wx_np = np.ascontiguousarray(wx_full).astype(bf)
    wh_np = np.ascontiguousarray(W_iouh.T).astype(bf)  # [256, 768]
    wf_np = np.ascontiguousarray(W_fh.T).astype(bf)  # [256, 256]

    xT = np.concatenate([inputs.T, np.ones((1, N_NODES), np.float32)], axis=0)
    xT = xT.astype(bf)  # [301, 65535]

    in_maps = []
    for j in range(NCORES):
        cols = np.empty([0], np.int64)
        segs = []
        for d in LEVELS:
            S = S_OF[d]
            g0 = (2**d - 1) + j * S
            segs.append(np.arange(g0, g0 + S))
        segs.append(np.arange(0, 7))
        cols = np.concatenate(segs)
        xcore = np.ascontiguousarray(xT[:, cols])
        in_maps.append({"xt": xcore, "wx": wx_np, "wh": wh_np, "wf": wf_np})
    return in_maps


def _postprocess(results):
    """Assemble [2, N, 256] from per-core [2, 2, 128, N_COLS] outputs."""
    full = np.empty((2, N_NODES, MEM), np.float32)
    for j in range(NCORES):
        r = results[j]["out"]  # [2(c/h), 2(ch), 128, N_COLS]
        for d in LEVELS:
            S = S_OF[d]
            g0 = (2**d - 1) + j * S
            off = OFF_OF[d]
            blk = r[:, :, :, off : off + S]  # [2,2,128,S]
            full[:, g0 : g0 + S, :] = blk.transpose(0, 3, 1, 2).reshape(2, S, MEM)
    r0 = results[0]["out"][:, :, :, TOP_COL0 : TOP_COL0 + 7]
    full[:, 0:7, :] = r0.transpose(0, 3, 1, 2).reshape(2, 7, MEM)
    return full


def kernel(**inputs):
    from concourse.bass_utils import run_bass_kernel_spmd

    nc = _get_program()
    in_maps = _preprocess(**inputs)
    res = run_bass_kernel_spmd(nc, in_maps, core_ids=list(range(NCORES)))
    _CACHE["last_result"] = res
    return _postprocess(res.results)
